# revision 1
# baseline (speedup 1.0000x reference)
"""Radon transform (bilinear grid-sample + row-sum) on 8 TRN2 NeuronCores.

Strategy: angle wedges are sharded across the 8 cores (rep-pure wedges: cores
0-3 process angles where |cos|>=|sin| on the identity frame, cores 4-7 the
rest on the transposed frame). On each core the 4 bilinear taps x 2 batches of
every sample are fetched with a single shared GPSIMD ap_gather index
(channel-shifted slab copies per 16-partition group; d=2 gathers the
horizontal tap pair; column parity handled by zeroing weights on wrong-parity
channels). Weighted taps are combined by DVE multiply + segment-reduce and a
TensorE ones-matmul partition-sum produces each sinogram column.

All gather indices / weights are input-independent and precomputed on host.
"""
import math
import os
import sys
from contextlib import ExitStack

import numpy as np

sys.path.insert(0, "/opt/trn_rl_repo")

import ml_dtypes  # noqa: E402

BF16 = ml_dtypes.bfloat16

# ─── geometry constants (hardcoded for 256x256, 180 angles, batch 2) ───
N_ANGLES = 180
IMG_SIZE = 256
BATCH = 2
S = int(math.ceil(math.sqrt(2.0) * IMG_SIZE))  # 363
PB = (S - IMG_SIZE) // 2                       # 53
FR = 520          # embedded frame size
EMB = 78          # embedding offset: tap rows/cols land in [3, 514]
NJ = 64           # j-rows per band-class slab
NBLK = 256        # d=2 blocks per j-row
WID = 512
NELEM_BLK = NJ * NBLK  # 16384 gather blocks per frame
NGROUP = 8
SLOTS = 23
SXPAD = 368       # 16*23, padded xg dimension
CHUNK_NXG = [80, 80, 80, 80, 48]

CORE_ANGLES = [
    list(range(0, 23)), list(range(23, 46)),
    list(range(135, 158)), list(range(158, 180)),
    list(range(46, 69)), list(range(69, 91)),
    list(range(91, 113)), list(range(113, 135)),
]
CORE_REP = [0, 0, 0, 0, 1, 1, 1, 1]


def _angle_tables(k):
    theta = np.float32(k) * np.float32(np.pi / N_ANGLES)
    c = np.cos(theta, dtype=np.float32)
    s = np.sin(theta, dtype=np.float32)
    lin = np.linspace(-1.0, 1.0, S, dtype=np.float32)
    x = lin[None, :]
    y = lin[:, None]
    gx = c * x + s * y
    gy = -s * x + c * y
    ix = ((gx + np.float32(1.0)) * np.float32(0.5) * np.float32(S - 1)).astype(np.float32)
    iy = ((gy + np.float32(1.0)) * np.float32(0.5) * np.float32(S - 1)).astype(np.float32)
    x0 = np.floor(ix)
    y0 = np.floor(iy)
    wx = ix - x0
    wy = iy - y0
    return (y0.astype(np.int32), x0.astype(np.int32),
            (1 - wx) * (1 - wy), wx * (1 - wy), (1 - wx) * wy, wx * wy, c, s)


def _plan_angle(k, lseg):
    """idx [8, nidx] int32 block indices, wch [8, 4, nidx, 2] f32 weights."""
    y0, x0, w00, w01, w10, w11, c, s = _angle_tables(k)
    y0e, x0e = y0 + EMB, x0 + EMB
    rep = 0 if abs(c) >= abs(s) else 1
    if rep == 0:
        r0, c0 = y0e, x0e
        wp = (w00, w01, w10, w11)  # cls = sr*2+par ; pair elem = dx
    else:
        r0, c0 = x0e, y0e
        wp = (w00, w10, w01, w11)  # sr = dx ; pair elem = dy
    gp = (r0 - 3) % 8
    j = (r0 - 3) // 8
    lx = c0 - 3
    par = lx % 2
    idx_flat = j * NBLK + lx // 2
    nidx = SXPAD * lseg
    idx = np.zeros((NGROUP, nidx), np.int32)
    wch = np.zeros((NGROUP, 4, nidx, 2), np.float32)
    for g in range(NGROUP):
        ygs, xgs = np.nonzero(gp == g)
        order = np.argsort(xgs, kind="stable")
        ygs, xgs = ygs[order], xgs[order]
        cnt = np.bincount(xgs, minlength=S)
        assert cnt.max() <= lseg, (k, g, cnt.max(), lseg)
        starts = np.concatenate([[0], np.cumsum(cnt)[:-1]])
        t = np.arange(len(xgs)) - starts[xgs]
        slot = xgs * lseg + t
        idx[g, slot] = idx_flat[ygs, xgs]
        pr = par[ygs, xgs]
        for sr in range(2):
            # pair elem e: weight of (sr, e): rep0: w[dy=sr][dx=e]; rep1: w[dy=e][dx=sr]
            we0 = wp[sr * 2 + 0][ygs, xgs]
            we1 = wp[sr * 2 + 1][ygs, xgs]
            for pp in range(2):
                cls = sr * 2 + pp
                m = (pr == pp).astype(np.float32)
                wch[g, cls, slot, 0] = we0 * m
                wch[g, cls, slot, 1] = we1 * m
    return rep, idx, wch


def _slot_lsegs():
    ls = np.zeros(SLOTS, np.int64)
    for ci in range(8):
        for si, k in enumerate(CORE_ANGLES[ci]):
            y0, x0, _, _, _, _, c, s = _angle_tables(k)
            rep = 0 if abs(c) >= abs(s) else 1
            r0 = (y0 if rep == 0 else x0) + EMB
            gp = (r0 - 3) % 8
            m = 0
            for g in range(NGROUP):
                m = max(m, int(np.bincount(np.nonzero(gp == g)[1], minlength=S).max()))
            ls[si] = max(ls[si], m)
    # make nidx = SXPAD*lseg multiple of 16 (SXPAD=368 = 16*23 -> always ok)
    return [int(v) for v in ls]


_PLAN_CACHE = {}


def _get_plan():
    if "plan" in _PLAN_CACHE:
        return _PLAN_CACHE["plan"]
    lsegs = _slot_lsegs()
    nidxs = [SXPAD * L for L in lsegs]
    # per-core packed idx blobs ([128, tot16] int16) and weight blobs
    # ([32, totw] bf16), plus chunk offset tables
    tot16 = sum(n // 16 for n in nidxs)
    totw = sum(n * 2 for n in nidxs)
    core_idx = []
    core_w = []
    for ci in range(8):
        idx_blob = np.zeros((128, tot16), np.int16)
        w_blob = np.zeros((32, totw), np.float32)
        o16 = 0
        ow = 0
        for si, k in enumerate(CORE_ANGLES[ci]):
            L = lsegs[si]
            n = nidxs[si]
            rep, idx, wch = _plan_angle(k, L)
            assert rep == CORE_REP[ci]
            for g in range(NGROUP):
                wrap = idx[g].reshape(n // 16, 16).T.astype(np.int16)  # [16, n/16]
                idx_blob[16 * g:16 * g + 16, o16:o16 + n // 16] = wrap
                for cls in range(4):
                    w_blob[g * 4 + cls, ow:ow + 2 * n] = wch[g, cls].reshape(-1)
            o16 += n // 16
            ow += 2 * n
        core_idx.append(idx_blob)
        core_w.append(w_blob.astype(BF16))
    sel = np.zeros((128, 2), np.float32)
    for p in range(128):
        cch = p % 16
        if cch % 2 == 0:
            sel[p, (cch % 4) // 2] = 1.0
    plan = dict(lsegs=lsegs, nidxs=nidxs, tot16=tot16, totw=totw,
                core_idx=core_idx, core_w=core_w, sel=sel)
    _PLAN_CACHE["plan"] = plan
    return plan


def _build_frame(image, rep):
    fr = np.zeros((BATCH, FR, FR), np.float32)
    img_s = np.zeros((BATCH, S, S), np.float32)
    img_s[:, PB:PB + IMG_SIZE, PB:PB + IMG_SIZE] = image[:, 0]
    fr[:, EMB:EMB + S, EMB:EMB + S] = img_s
    if rep:
        fr = np.ascontiguousarray(np.transpose(fr, (0, 2, 1)))
    return fr


def _build_slabs(frame):
    out = np.zeros((128, NELEM_BLK * 2), np.float32)
    for gp in range(NGROUP):
        for sr in range(2):
            rows = frame[:, 3 + gp + sr: 3 + gp + sr + 8 * NJ: 8, :]  # [B,64,520]
            for par in range(2):
                cols = rows[:, :, 3 + par: 3 + par + 2 * NBLK]  # [B,64,512]
                flat = cols.reshape(BATCH, -1)
                for b in range(BATCH):
                    for dup in range(2):
                        p = 16 * gp + (sr * 2 + par) * 4 + b * 2 + dup
                        out[p] = flat[b]
    return out


_PROG_CACHE = {}


def _build_program(plan):
    if "prog" in _PROG_CACHE:
        return _PROG_CACHE["prog"]
    import concourse.bass as bass
    import concourse.mybir as mybir
    from concourse import library_config

    lsegs = plan["lsegs"]
    nidxs = plan["nidxs"]
    maxcn = max(CHUNK_NXG) * max(lsegs)

    nc = bass.Bass()
    slab_d = nc.declare_dram_parameter("slab", [128, NELEM_BLK * 2],
                                       mybir.dt.bfloat16, isOutput=False)
    idx_d = nc.declare_dram_parameter("idx", [128, plan["tot16"]],
                                      mybir.dt.int16, isOutput=False)
    w_d = nc.declare_dram_parameter("w", [32, plan["totw"]],
                                    mybir.dt.bfloat16, isOutput=False)
    sel_d = nc.declare_dram_parameter("sel", [128, 2], mybir.dt.float32,
                                      isOutput=False)
    out_d = nc.declare_dram_parameter("out", [SLOTS, 2, SXPAD],
                                      mybir.dt.float32, isOutput=True)
    debug = bool(os.environ.get("RADON_DEBUG"))
    if debug:
        maxcn0 = max(CHUNK_NXG) * max(plan["lsegs"])
        dbg_g = nc.declare_dram_parameter("dbg_g", [128, maxcn0 * 2],
                                          mybir.dt.bfloat16, isOutput=True)
        dbg_w = nc.declare_dram_parameter("dbg_w", [128, maxcn0 * 2],
                                          mybir.dt.bfloat16, isOutput=True)
        dbg_p = nc.declare_dram_parameter("dbg_p", [128, maxcn0 * 2],
                                          mybir.dt.bfloat16, isOutput=True)
        dbg_r = nc.declare_dram_parameter("dbg_r", [128, SXPAD],
                                          mybir.dt.float32, isOutput=True)

    ctx = ExitStack()
    with ctx:
        slab_t = ctx.enter_context(nc.sbuf_tensor([128, NELEM_BLK * 2], mybir.dt.bfloat16))
        idx_t = ctx.enter_context(nc.sbuf_tensor([128, maxcn // 16], mybir.dt.int16))
        w_t = ctx.enter_context(nc.sbuf_tensor([128, maxcn * 2], mybir.dt.bfloat16))
        g_t = ctx.enter_context(nc.sbuf_tensor([128, maxcn * 2], mybir.dt.bfloat16))
        p_t = ctx.enter_context(nc.sbuf_tensor([128, maxcn * 2], mybir.dt.bfloat16))
        r_ts = [ctx.enter_context(nc.sbuf_tensor(f"r{i}", [128, SXPAD], mybir.dt.float32)) for i in range(2)]
        sel_t = ctx.enter_context(nc.sbuf_tensor([128, 2], mybir.dt.float32))
        vscr_t = ctx.enter_context(nc.sbuf_tensor([128, 2], mybir.dt.float32))
        ascr_t = ctx.enter_context(nc.sbuf_tensor([2, 2], mybir.dt.float32))
        sino_t = ctx.enter_context(nc.sbuf_tensor("sino", [2, SLOTS * SXPAD],
                                                   mybir.dt.float32))
        psum_ts = [ctx.enter_context(nc.psum_tensor(f"ps{i}", [2, SXPAD], mybir.dt.float32)) for i in range(2)]
        s_in = ctx.enter_context(nc.semaphore("s_in"))
        s_dma = ctx.enter_context(nc.semaphore("s_dma"))
        s_g = ctx.enter_context(nc.semaphore("s_g"))
        s_v = ctx.enter_context(nc.semaphore("s_v"))
        s_mm = ctx.enter_context(nc.semaphore("s_mm"))
        s_cp = ctx.enter_context(nc.semaphore("s_cp"))
        s_od = ctx.enter_context(nc.semaphore("s_od"))
        s_dbg = ctx.enter_context(nc.semaphore("s_dbg"))
        block = ctx.enter_context(nc.Block())

        # chunk schedule: list of (slot, ci, xoff, nxg, cn, o16, ow)
        chunks = []
        o16 = ow = 0
        for si in range(SLOTS):
            L = lsegs[si]
            xoff = 0
            for cidx, nxg in enumerate(CHUNK_NXG):
                cn = nxg * L
                chunks.append(dict(si=si, cidx=cidx, L=L, xoff=xoff, nxg=nxg,
                                   cn=cn, o16=o16, ow=ow))
                xoff += nxg
                o16 += cn // 16
                ow += 2 * cn
        nchunks = len(chunks)

        @block.sync
        def _(sync):
            sync.dma_start(out=slab_t[:], in_=slab_d[:]).then_inc(s_in, 16)
            sync.dma_start(out=sel_t[:], in_=sel_d[:]).then_inc(s_in, 16)
            for n, ch in enumerate(chunks):
                # reuse guards: idx_t read by gather n-1; w_t read by vector n-1
                if n > 0:
                    sync.wait_ge(s_g, n)
                    sync.wait_ge(s_v, n)
                if debug and n == len(CHUNK_NXG):
                    sync.wait_ge(s_v, len(CHUNK_NXG))
                    sync.dma_start(out=dbg_r[:], in_=r_ts[0][:]).then_inc(s_dbg, 16)
                sync.dma_start(
                    out=idx_t[:, :ch["cn"] // 16],
                    in_=idx_d[:, ch["o16"]:ch["o16"] + ch["cn"] // 16],
                ).then_inc(s_dma, 16)
                wsrc = (w_d[:, ch["ow"]:ch["ow"] + 2 * ch["cn"]]
                        .unsqueeze(1).broadcast_to([32, 4, 2 * ch["cn"]]))
                sync.dma_start(out=w_t[:, :2 * ch["cn"]], in_=wsrc).then_inc(s_dma, 16)

        @block.gpsimd
        def _(g):
            g.load_library(library_config.ap_gather)
            g.wait_ge(s_in, 32)
            # warmup + startup barrier with VALID indices (chunk 0 already
            # DMA'd): burn ~200us so all preamble DMA descriptor streams
            # (slab) have fully landed before the real gathers
            g.wait_ge(s_dma, 32)
            ch0 = chunks[0]
            for _ in range(2):
                g.ap_gather(
                    g_t[:, :2 * ch0["cn"]].rearrange("p (n d) -> p n d", d=2),
                    slab_t[:].rearrange("p (n d) -> p n d", d=2),
                    idx_t[:, :ch0["cn"] // 16],
                    channels=128, num_elems=NELEM_BLK, d=2, num_idxs=ch0["cn"],
                )
            for n, ch in enumerate(chunks):
                g.wait_ge(s_dma, 32 * (n + 1))
                if n > 0:
                    g.wait_ge(s_v, n)  # g_t consumed by vector of chunk n-1
                g.ap_gather(
                    g_t[:, :2 * ch["cn"]].rearrange("p (n d) -> p n d", d=2),
                    slab_t[:].rearrange("p (n d) -> p n d", d=2),
                    idx_t[:, :ch["cn"] // 16],
                    channels=128, num_elems=NELEM_BLK, d=2, num_idxs=ch["cn"],
                ).then_inc(s_g, 1)

        @block.vector
        def _(v):
            for n, ch in enumerate(chunks):
                v.wait_ge(s_g, n + 1)
                if ch["cidx"] == 0 and ch["si"] > 1:
                    v.wait_ge(s_mm, ch["si"] - 1)  # r_t buffer consumed by matmul
                if debug and ch["si"] == 2 and ch["cidx"] == 0:
                    v.wait_ge(s_dbg, 16)  # r_ts[0] dumped before slot-2 overwrites
                v.tensor_mul(p_t[:, :2 * ch["cn"]], g_t[:, :2 * ch["cn"]],
                             w_t[:, :2 * ch["cn"]])
                rdst = r_ts[ch["si"] % 2]
                v.tensor_reduce(
                    out=rdst[:, ch["xoff"]:ch["xoff"] + ch["nxg"]],
                    in_=p_t[:, :2 * ch["cn"]].rearrange(
                        "p (x l) -> p x l", l=2 * ch["L"]),
                    axis=mybir.AxisListType.X,
                    op=mybir.AluOpType.add,
                )
                # drain fence: DVE issues in order after pipe empties, so this
                # inc observes the reduce's writes as complete
                v.tensor_copy(vscr_t[:, :1],
                              rdst[:, ch["xoff"]:ch["xoff"] + 1]).then_inc(s_v, 1)

        @block.tensor
        def _(t):
            for si in range(SLOTS):
                t.wait_ge(s_v, (si + 1) * len(CHUNK_NXG))
                if si > 1:
                    t.wait_ge(s_cp, si - 1)  # psum buffer consumed by scalar copy
                t.matmul(psum_ts[si % 2][:], sel_t[:], r_ts[si % 2][:],
                         start=True, stop=True).then_inc(s_mm, 1)

        @block.scalar
        def _(sc):
            for si in range(SLOTS):
                sc.wait_ge(s_mm, si + 1)
                sc.copy(sino_t[:, si * SXPAD:(si + 1) * SXPAD], psum_ts[si % 2][:])
                sc.copy(ascr_t[:, :1],
                        sino_t[:, si * SXPAD:si * SXPAD + 1]).then_inc(s_cp, 1)
            sc.wait_ge(s_cp, SLOTS)
            sc.dma_start(out=out_d.rearrange("s b x -> b s x"),
                         in_=sino_t[:].rearrange("b (s x) -> b s x", x=SXPAD)
                         ).then_inc(s_od, 16)
            sc.wait_ge(s_od, 16)

    mybir.codegen_inst_isa_subclasses(nc)
    _PROG_CACHE["prog"] = nc
    return nc


def kernel(image):
    image = np.asarray(image, np.float32)
    assert image.shape == (BATCH, 1, IMG_SIZE, IMG_SIZE)
    plan = _get_plan()
    nc = _build_program(plan)

    from concourse.bass_utils import run_bass_kernel_spmd

    in_maps = []
    for ci in range(8):
        frame = _build_frame(image, CORE_REP[ci])
        slab = _build_slabs(frame).astype(BF16)
        in_maps.append({
            "slab": slab,
            "idx": plan["core_idx"][ci],
            "w": plan["core_w"][ci],
            "sel": plan["sel"],
        })

    trace = bool(os.environ.get("RADON_TRACE"))
    if trace:
        _install_profhook()
    res = run_bass_kernel_spmd(nc, in_maps, list(range(8)), trace=trace)
    if trace:
        kernel.last_exec_time_ns = res.exec_time_ns

    sino = np.zeros((BATCH, 1, S, N_ANGLES), np.float32)
    for ci in range(8):
        o = res.results[ci]["out"]  # [SLOTS, 2, SXPAD]
        for si, k in enumerate(CORE_ANGLES[ci]):
            sino[:, 0, :, k] = o[si, :, :S]
    return sino


def _install_profhook():
    import types
    if "antenv.axon_hooks" in sys.modules:
        return
    try:
        from trn_agent_boot.trn_boot import _ntff_profile_via_ctypes
        hook = _ntff_profile_via_ctypes("/opt/axon/libaxon_pjrt.so")
    except Exception:
        hook = None
    mod = types.ModuleType("antenv.axon_hooks")
    mod._hook = hook
    mod.set_axon_ntff_profile_hook = lambda h: setattr(mod, "_hook", h)
    mod.get_axon_ntff_profile_hook = lambda: mod._hook
    sys.modules["antenv.axon_hooks"] = mod
    import antenv
    antenv.axon_hooks = mod


if __name__ == "__main__":
    img = np.load("/tmp/ref_image.npy")
    out = kernel(image=img)
    exp = np.load("/tmp/ref_expected.npy")
    err = np.linalg.norm(out - exp) / np.linalg.norm(exp)
    print("kernel rel err:", err)



# revision 2
# speedup vs baseline: 9.4977x; 9.4977x over previous
"""Radon transform (bilinear grid-sample + row-sum) on 8 TRN2 NeuronCores.

Angle wedges sharded across 8 cores (rep-pure wedges: identity frame for
|cos|>=|sin|, transposed frame otherwise). Per core, per angle: every
detector ray x is decomposed into 32 8-row blocks of the content region;
one GPSIMD ap_gather index per (ray, block) fetches a 16-wide column window
(hop-4 aligned, overlapping slab storage) that covers all bilinear taps of
the ray in that block. The 16 channels of each Q7 core hold the 8 row
phases x 2 batches, so every gathered lane is useful. DVE multiplies by
precomputed tap weights and segment-reduces each ray (32 blocks x 16 taps
= 512 elems); a TensorE ones-style matmul sums the 128 partitions into
(group, batch) sinogram rows. All indices/weights are input-independent
and precomputed on host.
"""
import math
import os
import sys
from contextlib import ExitStack

import numpy as np

sys.path.insert(0, "/opt/trn_rl_repo")

import ml_dtypes  # noqa: E402

BF16 = ml_dtypes.bfloat16

# ─── geometry constants (hardcoded for 256x256, 180 angles, batch 2) ───
N_ANGLES = 180
IMG_SIZE = 256
BATCH = 2
S = int(math.ceil(math.sqrt(2.0) * IMG_SIZE))  # 363
PB = (S - IMG_SIZE) // 2                       # 53
ROFF = 53         # slab row/col origin = content origin
HOP = 4           # window alignment granularity
D = 16            # window width (bf16 elems per gather block)
NH = 64           # hop positions per slab row
NJ = 32           # 8-row blocks covering the 256 content rows
NELEM = NJ * NH   # 2048 gather blocks per slab partition
NMAX = NJ         # block slots per ray (j used directly)
NRAYS_G = 46      # rays per Q7 group (363 = 8*45+3, padded)
NTOT = NRAYS_G * NMAX          # 1472 indices per slot per group
SEG = NMAX * D                 # 512 elems reduced per ray
SLOTS = 23
CHUNK_NR = [16, 16, 14]        # rays per chunk
NCHUNK = len(CHUNK_NR)

CORE_ANGLES = [
    list(range(0, 23)), list(range(23, 46)),
    list(range(135, 158)), list(range(158, 180)),
    list(range(46, 69)), list(range(69, 91)),
    list(range(91, 113)), list(range(113, 135)),
]
CORE_REP = [0, 0, 0, 0, 1, 1, 1, 1]


def _angle_taps(k):
    """Content-region bilinear taps in rep-frame coords.

    Returns rep, xs (detector ray), j (8-row block), phi (row phase),
    cc (slab col = col-ROFF), ws (f32 weight)."""
    th = np.float32(k) * np.float32(np.pi / N_ANGLES)
    c = np.cos(th, dtype=np.float32)
    s = np.sin(th, dtype=np.float32)
    lin = np.linspace(-1.0, 1.0, S, dtype=np.float32)
    gx = c * lin[None, :] + s * lin[:, None]
    gy = -s * lin[None, :] + c * lin[:, None]
    ix = (gx + np.float32(1)) * np.float32(0.5) * np.float32(S - 1)
    iy = (gy + np.float32(1)) * np.float32(0.5) * np.float32(S - 1)
    x0 = np.floor(ix).astype(np.int64)
    y0 = np.floor(iy).astype(np.int64)
    wx = (ix - x0).astype(np.float32)
    wy = (iy - y0).astype(np.float32)
    rep = 0 if abs(c) >= abs(s) else 1
    rows_l, cols_l, ws_l, xs_l = [], [], [], []
    for dy in (0, 1):
        for dx in (0, 1):
            r = y0 + dy
            q = x0 + dx
            w = (wy if dy else 1 - wy) * (wx if dx else 1 - wx)
            m = ((r >= PB) & (r < PB + IMG_SIZE)
                 & (q >= PB) & (q < PB + IMG_SIZE) & (w != 0))
            _, xx = np.nonzero(m)
            rows_l.append(r[m])
            cols_l.append(q[m])
            ws_l.append(w[m])
            xs_l.append(xx)
    rows = np.concatenate(rows_l)
    cols = np.concatenate(cols_l)
    ws = np.concatenate(ws_l)
    xs = np.concatenate(xs_l)
    if rep:
        rows, cols = cols, rows
    j = (rows - ROFF) // 8
    phi = (rows - ROFF) % 8
    cc = cols - ROFF
    return rep, xs, j, phi, cc, ws


def _plan_angle(k):
    """idx [368, NJ] int16 block ids; wt [368, NJ, 8, D] f32 weights."""
    rep, xs, j, phi, cc, ws = _angle_taps(k)
    qmin = np.full((368, NJ), 255, np.int64)
    np.minimum.at(qmin, (xs, j), cc)
    hq = np.clip(qmin // HOP, 0, NH - D // HOP)
    wt = np.zeros((368, NJ, 8, D), np.float32)
    kk = cc - HOP * hq[xs, j]
    assert kk.min() >= 0 and kk.max() < D, (k, kk.min(), kk.max())
    np.add.at(wt, (xs, j, phi, kk), ws)
    present = np.zeros((368, NJ), bool)
    present[xs, j] = True
    idx = np.where(present, j[0] * 0 + np.arange(NJ)[None, :] * NH + hq, 0)
    return rep, idx.astype(np.int16), wt


_PLAN_CACHE = {}


def _get_plan():
    if "plan" in _PLAN_CACHE:
        return _PLAN_CACHE["plan"]
    core_idx = []
    core_w = []
    for ci in range(8):
        idx_blob = np.zeros((128, SLOTS * (NTOT // 16)), np.int16)
        w_blob = np.zeros((64, SLOTS * NTOT * D), np.float32)
        for si, k in enumerate(CORE_ANGLES[ci]):
            rep, idx, wt = _plan_angle(k)
            assert rep == CORE_REP[ci]
            # idx[x, j] -> group g = x%8, xi = x//8, n = xi*NJ + j
            ig = idx.reshape(NRAYS_G, 8, NJ).transpose(1, 0, 2).reshape(8, NTOT)
            wrap = ig.reshape(8, NTOT // 16, 16)
            for g in range(8):
                idx_blob[16 * g:16 * g + 16,
                         si * (NTOT // 16):(si + 1) * (NTOT // 16)] = wrap[g].T
            # wt[x, j, phi, k] -> row 8g+phi, col (si*NTOT + xi*NJ + j)*D + k
            wg = (wt.reshape(NRAYS_G, 8, NJ, 8, D)
                  .transpose(1, 3, 0, 2, 4).reshape(64, NTOT * D))
            w_blob[:, si * NTOT * D:(si + 1) * NTOT * D] = wg
        core_idx.append(idx_blob)
        core_w.append(w_blob.astype(BF16))
    sel = np.zeros((128, 16), np.float32)
    for p in range(128):
        sel[p, 2 * (p // 16) + (p % 2)] = 1.0
    plan = dict(core_idx=core_idx, core_w=core_w, sel=sel)
    _PLAN_CACHE["plan"] = plan
    return plan


def _build_slab(image, rep):
    """[16, NELEM*D] bf16: channel 2*phi+b holds hop-4 overlapping windows
    of content rows ROFF+8j+phi."""
    fr = np.zeros((BATCH, S, S), np.float32)
    fr[:, PB:PB + IMG_SIZE, PB:PB + IMG_SIZE] = image[:, 0]
    if rep:
        fr = np.ascontiguousarray(np.transpose(fr, (0, 2, 1)))
    out = np.zeros((16, NELEM * D), np.float32)
    span = HOP * (NH - 1) + D  # 268 cols
    for phi in range(8):
        rows = fr[:, ROFF + phi: ROFF + phi + 8 * NJ: 8, ROFF:ROFF + span]
        win = np.lib.stride_tricks.sliding_window_view(rows, D, axis=2)
        win = win[:, :, ::HOP, :]  # [B, NJ, NH, D]
        assert win.shape == (BATCH, NJ, NH, D)
        for b in range(BATCH):
            out[2 * phi + b] = win[b].reshape(-1)
    return out.astype(BF16)


_PROG_CACHE = {}


def _build_program():
    if "prog" in _PROG_CACHE:
        return _PROG_CACHE["prog"]
    import concourse.bass as bass
    import concourse.mybir as mybir
    from concourse import library_config

    nc = bass.Bass()
    slab_d = nc.declare_dram_parameter("slab", [16, NELEM * D],
                                       mybir.dt.bfloat16, isOutput=False)
    idx_d = nc.declare_dram_parameter("idx", [128, SLOTS * (NTOT // 16)],
                                      mybir.dt.int16, isOutput=False)
    w_d = nc.declare_dram_parameter("w", [64, SLOTS * NTOT * D],
                                    mybir.dt.bfloat16, isOutput=False)
    sel_d = nc.declare_dram_parameter("sel", [128, 16], mybir.dt.float32,
                                      isOutput=False)
    out_d = nc.declare_dram_parameter("out", [16, SLOTS * NRAYS_G],
                                      mybir.dt.float32, isOutput=True)

    ctx = ExitStack()
    with ctx:
        slab_t = ctx.enter_context(
            nc.sbuf_tensor([128, NELEM * D], mybir.dt.bfloat16))
        idx_t = ctx.enter_context(
            nc.sbuf_tensor([128, SLOTS * (NTOT // 16)], mybir.dt.int16))
        wt_ts = [ctx.enter_context(
            nc.sbuf_tensor(f"wt{i}", [128, 512 * D], mybir.dt.bfloat16))
            for i in range(2)]
        win_ts = [ctx.enter_context(
            nc.sbuf_tensor(f"win{i}", [128, 512 * D], mybir.dt.bfloat16))
            for i in range(2)]
        prod_t = ctx.enter_context(
            nc.sbuf_tensor([128, 512 * D], mybir.dt.bfloat16))
        red_ts = [ctx.enter_context(
            nc.sbuf_tensor(f"red{i}", [128, NRAYS_G], mybir.dt.float32))
            for i in range(2)]
        sel_t = ctx.enter_context(nc.sbuf_tensor([128, 16], mybir.dt.float32))
        vscr_t = ctx.enter_context(nc.sbuf_tensor([128, 2], mybir.dt.float32))
        ascr_t = ctx.enter_context(nc.sbuf_tensor([16, 2], mybir.dt.float32))
        sino_t = ctx.enter_context(
            nc.sbuf_tensor("sino", [16, SLOTS * NRAYS_G], mybir.dt.float32))
        psum_ts = [ctx.enter_context(
            nc.psum_tensor(f"ps{i}", [16, NRAYS_G], mybir.dt.float32))
            for i in range(2)]
        s_in = ctx.enter_context(nc.semaphore("s_in"))
        s_dma = ctx.enter_context(nc.semaphore("s_dma"))
        s_g = ctx.enter_context(nc.semaphore("s_g"))
        s_v = ctx.enter_context(nc.semaphore("s_v"))
        s_mm = ctx.enter_context(nc.semaphore("s_mm"))
        s_cp = ctx.enter_context(nc.semaphore("s_cp"))
        s_od = ctx.enter_context(nc.semaphore("s_od"))
        block = ctx.enter_context(nc.Block())

        # chunk schedule: (slot, cidx, r0, nr, cn, idx col offset, w offset)
        chunks = []
        for si in range(SLOTS):
            r0 = 0
            for cidx, nr in enumerate(CHUNK_NR):
                cn = nr * NMAX
                chunks.append(dict(
                    si=si, cidx=cidx, r0=r0, nr=nr, cn=cn,
                    o16=si * (NTOT // 16) + r0 * NMAX // 16,
                    ow=(si * NTOT + r0 * NMAX) * D))
                r0 += nr

        @block.sync
        def _(sync):
            slab_src = (slab_d[:].unsqueeze(0)
                        .broadcast_to([8, 16, NELEM * D]))
            sync.dma_start(out=slab_t[:], in_=slab_src).then_inc(s_in, 16)
            sync.dma_start(out=idx_t[:], in_=idx_d[:]).then_inc(s_in, 16)
            sync.dma_start(out=sel_t[:], in_=sel_d[:]).then_inc(s_in, 16)
            for n, ch in enumerate(chunks):
                if n > 1:
                    sync.wait_ge(s_v, n - 1)  # wt buffer consumed
                wsrc = (w_d[:, ch["ow"]:ch["ow"] + ch["cn"] * D]
                        .unsqueeze(1).broadcast_to([64, 2, ch["cn"] * D]))
                sync.dma_start(out=wt_ts[n % 2][:, :ch["cn"] * D],
                               in_=wsrc).then_inc(s_dma, 16)

        @block.gpsimd
        def _(g):
            g.load_library(library_config.ap_gather)
            g.wait_ge(s_in, 48)
            g.wait_ge(s_dma, 16)
            ch0 = chunks[0]
            for _ in range(2):  # warmup + DMA landing barrier
                g.ap_gather(
                    win_ts[0][:, :ch0["cn"] * D].rearrange(
                        "p (n d) -> p n d", d=D),
                    slab_t[:].rearrange("p (n d) -> p n d", d=D),
                    idx_t[:, ch0["o16"]:ch0["o16"] + ch0["cn"] // 16],
                    channels=128, num_elems=NELEM, d=D, num_idxs=ch0["cn"],
                )
            for n, ch in enumerate(chunks):
                if n > 1:
                    g.wait_ge(s_v, n - 1)  # win buffer consumed
                g.ap_gather(
                    win_ts[n % 2][:, :ch["cn"] * D].rearrange(
                        "p (n d) -> p n d", d=D),
                    slab_t[:].rearrange("p (n d) -> p n d", d=D),
                    idx_t[:, ch["o16"]:ch["o16"] + ch["cn"] // 16],
                    channels=128, num_elems=NELEM, d=D, num_idxs=ch["cn"],
                ).then_inc(s_g, 1)

        @block.vector
        def _(v):
            for n, ch in enumerate(chunks):
                v.wait_ge(s_g, n + 1)
                v.wait_ge(s_dma, 16 * (n + 1))
                if ch["cidx"] == 0 and ch["si"] > 1:
                    v.wait_ge(s_mm, ch["si"] - 1)  # red buffer consumed
                v.tensor_mul(prod_t[:, :ch["cn"] * D],
                             win_ts[n % 2][:, :ch["cn"] * D],
                             wt_ts[n % 2][:, :ch["cn"] * D])
                v.tensor_reduce(
                    out=red_ts[ch["si"] % 2][:, ch["r0"]:ch["r0"] + ch["nr"]],
                    in_=prod_t[:, :ch["cn"] * D].rearrange(
                        "p (x l) -> p x l", l=SEG),
                    axis=mybir.AxisListType.X,
                    op=mybir.AluOpType.add,
                )
                # drain fence: DVE in-order; this inc observes the reduce
                v.tensor_copy(
                    vscr_t[:, :1],
                    red_ts[ch["si"] % 2][:, ch["r0"]:ch["r0"] + 1],
                ).then_inc(s_v, 1)

        @block.tensor
        def _(t):
            for si in range(SLOTS):
                t.wait_ge(s_v, (si + 1) * NCHUNK)
                if si > 1:
                    t.wait_ge(s_cp, si - 1)  # psum consumed by scalar copy
                t.matmul(psum_ts[si % 2][:], sel_t[:], red_ts[si % 2][:],
                         start=True, stop=True).then_inc(s_mm, 1)

        @block.scalar
        def _(sc):
            for si in range(SLOTS):
                sc.wait_ge(s_mm, si + 1)
                sc.copy(sino_t[:, si * NRAYS_G:(si + 1) * NRAYS_G],
                        psum_ts[si % 2][:])
                sc.copy(ascr_t[:, :1],
                        sino_t[:, si * NRAYS_G:si * NRAYS_G + 1]
                        ).then_inc(s_cp, 1)
            sc.wait_ge(s_cp, SLOTS)
            sc.dma_start(out=out_d[:], in_=sino_t[:]).then_inc(s_od, 16)
            sc.wait_ge(s_od, 16)

    mybir.codegen_inst_isa_subclasses(nc)
    _PROG_CACHE["prog"] = nc
    return nc


def kernel(image):
    image = np.asarray(image, np.float32)
    assert image.shape == (BATCH, 1, IMG_SIZE, IMG_SIZE)
    plan = _get_plan()
    nc = _build_program()

    from concourse.bass_utils import run_bass_kernel_spmd

    slabs = {rep: _build_slab(image, rep) for rep in (0, 1)}
    in_maps = []
    for ci in range(8):
        in_maps.append({
            "slab": slabs[CORE_REP[ci]],
            "idx": plan["core_idx"][ci],
            "w": plan["core_w"][ci],
            "sel": plan["sel"],
        })

    trace = bool(os.environ.get("RADON_TRACE"))
    if trace:
        _install_profhook()
    res = run_bass_kernel_spmd(nc, in_maps, list(range(8)), trace=trace)
    if trace:
        kernel.last_exec_time_ns = res.exec_time_ns

    sino = np.zeros((BATCH, 1, S, N_ANGLES), np.float32)
    for ci in range(8):
        o = res.results[ci]["out"]  # [16, SLOTS*NRAYS_G]
        for si, k in enumerate(CORE_ANGLES[ci]):
            v = o[:, si * NRAYS_G:(si + 1) * NRAYS_G].reshape(8, 2, NRAYS_G)
            # x = 8*xi + g
            full = v.transpose(1, 2, 0).reshape(BATCH, NRAYS_G * 8)
            sino[:, 0, :, k] = full[:, :S]
    return sino


def _install_profhook():
    import types
    if "antenv.axon_hooks" in sys.modules:
        return
    try:
        from trn_agent_boot.trn_boot import _ntff_profile_via_ctypes
        hook = _ntff_profile_via_ctypes("/opt/axon/libaxon_pjrt.so")
    except Exception:
        hook = None
    mod = types.ModuleType("antenv.axon_hooks")
    mod._hook = hook
    mod.set_axon_ntff_profile_hook = lambda h: setattr(mod, "_hook", h)
    mod.get_axon_ntff_profile_hook = lambda: mod._hook
    sys.modules["antenv.axon_hooks"] = mod
    import antenv
    antenv.axon_hooks = mod


if __name__ == "__main__":
    img = np.load("/tmp/ref_image.npy")
    out = kernel(image=img)
    exp = np.load("/tmp/ref_expected.npy")
    err = np.linalg.norm(out - exp) / np.linalg.norm(exp)
    print("kernel rel err:", err)


# revision 11
# speedup vs baseline: 11.1646x; 1.1755x over previous
"""Radon transform (bilinear grid-sample + row-sum) on 8 TRN2 NeuronCores.

Angle wedges sharded across 8 cores (rep-pure wedges: identity frame for
|cos|>=|sin|, transposed frame otherwise). Per core, per angle: every
detector ray x is decomposed into 32 8-row blocks of the content region;
one GPSIMD ap_gather index per (ray, block) fetches a 16-wide column window
(hop-4 aligned, overlapping slab storage) that covers all bilinear taps of
the ray in that block. The 16 channels of each Q7 core hold the 8 row
phases x 2 batches, so every gathered lane is useful. DVE multiplies by
precomputed tap weights and segment-reduces each ray (32 blocks x 16 taps
= 512 elems); a TensorE ones-style matmul sums the 128 partitions into
(group, batch) sinogram rows. All indices/weights are input-independent
and precomputed on host.
"""
import math
import os
import sys
from contextlib import ExitStack

import numpy as np

sys.path.insert(0, "/opt/trn_rl_repo")

import ml_dtypes  # noqa: E402

BF16 = ml_dtypes.bfloat16

# ─── geometry constants (hardcoded for 256x256, 180 angles, batch 2) ───
N_ANGLES = 180
IMG_SIZE = 256
BATCH = 2
S = int(math.ceil(math.sqrt(2.0) * IMG_SIZE))  # 363
PB = (S - IMG_SIZE) // 2                       # 53
ROFF = 53         # slab row/col origin = content origin
HOP = 4           # window alignment granularity
D = 16            # window width (bf16 elems per gather block)
NH = 64           # hop positions per slab row
NJ = 32           # 8-row blocks covering the 256 content rows
NELEM = NJ * NH   # 2048 gather blocks per slab partition
NMAX = NJ         # block slots per ray-pair (j used directly)
NXI = 23          # ray-pairs per Q7 group (x = 16*xi + 2g + e)
NRAYS_G = 2 * NXI              # 46 ray columns per group (e-major)
NTOT = NXI * NMAX              # 736 indices per slot per group
SEG = NMAX * D                 # 512 elems reduced per (ray, e)
SLOTS = 23
CHUNK_NXI = [12, 11]           # ray-pairs per chunk
NCHUNK = len(CHUNK_NXI)

CORE_ANGLES = [
    list(range(0, 23)), list(range(23, 46)),
    list(range(135, 158)), list(range(158, 180)),
    list(range(46, 69)), list(range(69, 91)),
    list(range(91, 113)), list(range(113, 135)),
]
CORE_REP = [0, 0, 0, 0, 1, 1, 1, 1]


def _angle_taps(k):
    """Content-region bilinear taps in rep-frame coords.

    Returns rep, xs (detector ray), j (8-row block), phi (row phase),
    cc (slab col = col-ROFF), ws (f32 weight)."""
    th = np.float32(k) * np.float32(np.pi / N_ANGLES)
    c = np.cos(th, dtype=np.float32)
    s = np.sin(th, dtype=np.float32)
    lin = np.linspace(-1.0, 1.0, S, dtype=np.float32)
    gx = c * lin[None, :] + s * lin[:, None]
    gy = -s * lin[None, :] + c * lin[:, None]
    ix = (gx + np.float32(1)) * np.float32(0.5) * np.float32(S - 1)
    iy = (gy + np.float32(1)) * np.float32(0.5) * np.float32(S - 1)
    x0 = np.floor(ix).astype(np.int64)
    y0 = np.floor(iy).astype(np.int64)
    wx = (ix - x0).astype(np.float32)
    wy = (iy - y0).astype(np.float32)
    rep = 0 if abs(c) >= abs(s) else 1
    rows_l, cols_l, ws_l, xs_l = [], [], [], []
    for dy in (0, 1):
        for dx in (0, 1):
            r = y0 + dy
            q = x0 + dx
            w = (wy if dy else 1 - wy) * (wx if dx else 1 - wx)
            m = ((r >= PB) & (r < PB + IMG_SIZE)
                 & (q >= PB) & (q < PB + IMG_SIZE) & (w != 0))
            _, xx = np.nonzero(m)
            rows_l.append(r[m])
            cols_l.append(q[m])
            ws_l.append(w[m])
            xs_l.append(xx)
    rows = np.concatenate(rows_l)
    cols = np.concatenate(cols_l)
    ws = np.concatenate(ws_l)
    xs = np.concatenate(xs_l)
    if rep:
        rows, cols = cols, rows
    j = (rows - ROFF) // 8
    phi = (rows - ROFF) % 8
    cc = cols - ROFF
    return rep, xs, j, phi, cc, ws


def _plan_angle(k):
    """Pair layout: pair p2 = x//2 (g = p2%8, xi = p2//8), e = x%2.

    Returns idx [184, NJ] int16 block ids per pair, and
    wt [2, 184, NJ, 8, D] f32 weights (e-major)."""
    rep, xs, j, phi, cc, ws = _angle_taps(k)
    p2 = xs // 2
    e = xs % 2
    qmin = np.full((184, NJ), 10 ** 6, np.int64)
    np.minimum.at(qmin, (p2, j), cc)
    hq = np.clip(qmin // HOP, 0, NH - D // HOP)
    wt = np.zeros((2, 184, NJ, 8, D), np.float32)
    kk = cc - HOP * hq[p2, j]
    assert kk.min() >= 0 and kk.max() < D, (k, kk.min(), kk.max())
    np.add.at(wt, (e, p2, j, phi, kk), ws)
    present = np.zeros((184, NJ), bool)
    present[p2, j] = True
    idx = np.where(present, np.arange(NJ)[None, :] * NH + hq, 0)
    return rep, idx.astype(np.int16), wt


_PLAN_CACHE = {}


def _get_plan():
    if "plan" in _PLAN_CACHE:
        return _PLAN_CACHE["plan"]
    slot_w = SLOTS * 2 * NTOT * D
    core_idx = []
    core_w = []
    for ci in range(8):
        idx_blob = np.zeros((128, SLOTS * (NTOT // 16)), np.int16)
        w_blob = np.zeros((64, slot_w // SLOTS * SLOTS), np.float32)
        for si, k in enumerate(CORE_ANGLES[ci]):
            rep, idx, wt = _plan_angle(k)
            assert rep == CORE_REP[ci]
            # idx[p2, j] -> group g = p2%8, n = xi*NJ + j
            ig = idx.reshape(NXI, 8, NJ).transpose(1, 0, 2).reshape(8, NTOT)
            wrap = ig.reshape(8, NTOT // 16, 16)
            for g in range(8):
                idx_blob[16 * g:16 * g + 16,
                         si * (NTOT // 16):(si + 1) * (NTOT // 16)] = wrap[g].T
            # wt[e, p2, j, phi, k] -> row 8g+phi,
            # chunk-major cols: [chunk][e][xi_local][j][k]
            wg = (wt.reshape(2, NXI, 8, NJ, 8, D)
                  .transpose(2, 4, 0, 1, 3, 5))  # [g, phi, e, xi, j, k]
            base = si * 2 * NTOT * D
            xi0 = 0
            for nxi in CHUNK_NXI:
                sz = 2 * nxi * NJ * D
                blockw = (wg[:, :, :, xi0:xi0 + nxi]
                          .reshape(64, sz))
                w_blob[:, base:base + sz] = blockw
                base += sz
                xi0 += nxi
        core_idx.append(idx_blob)
        core_w.append(w_blob.astype(BF16))
    sel = np.zeros((128, 16), np.float32)
    for p in range(128):
        sel[p, 2 * (p // 16) + (p % 2)] = 1.0
    plan = dict(core_idx=core_idx, core_w=core_w, sel=sel)
    _PLAN_CACHE["plan"] = plan
    return plan


def _build_slab(image, rep):
    """[16, NELEM*D] bf16: channel 2*phi+b holds hop-4 overlapping windows
    of content rows ROFF+8j+phi."""
    fr = np.zeros((BATCH, S, S), np.float32)
    fr[:, PB:PB + IMG_SIZE, PB:PB + IMG_SIZE] = image[:, 0]
    if rep:
        fr = np.ascontiguousarray(np.transpose(fr, (0, 2, 1)))
    out = np.zeros((16, NELEM * D), np.float32)
    span = HOP * (NH - 1) + D  # 268 cols
    for phi in range(8):
        rows = fr[:, ROFF + phi: ROFF + phi + 8 * NJ: 8, ROFF:ROFF + span]
        win = np.lib.stride_tricks.sliding_window_view(rows, D, axis=2)
        win = win[:, :, ::HOP, :]  # [B, NJ, NH, D]
        assert win.shape == (BATCH, NJ, NH, D)
        for b in range(BATCH):
            out[2 * phi + b] = win[b].reshape(-1)
    return out.astype(BF16)


_PROG_CACHE = {}


def _build_program():
    if "prog" in _PROG_CACHE:
        return _PROG_CACHE["prog"]
    import concourse.bass as bass
    import concourse.mybir as mybir
    from concourse import library_config

    nc = bass.Bass()
    slab_d = nc.declare_dram_parameter("slab", [16, NELEM * D],
                                       mybir.dt.bfloat16, isOutput=False)
    idx_d = nc.declare_dram_parameter("idx", [128, SLOTS * (NTOT // 16)],
                                      mybir.dt.int16, isOutput=False)
    w_d = nc.declare_dram_parameter("w", [64, SLOTS * 2 * NTOT * D],
                                    mybir.dt.bfloat16, isOutput=False)
    sel_d = nc.declare_dram_parameter("sel", [128, 16], mybir.dt.float32,
                                      isOutput=False)
    out_d = nc.declare_dram_parameter("out", [16, SLOTS * NRAYS_G],
                                      mybir.dt.float32, isOutput=True)

    ctx = ExitStack()
    with ctx:
        slab_t = ctx.enter_context(
            nc.sbuf_tensor([128, NELEM * D], mybir.dt.bfloat16))
        idx_t = ctx.enter_context(
            nc.sbuf_tensor([128, SLOTS * (NTOT // 16)], mybir.dt.int16))
        maxw = max(CHUNK_NXI) * NJ  # windows per chunk
        wt_ts = [ctx.enter_context(
            nc.sbuf_tensor(f"wt{i}", [128, 2 * maxw * D], mybir.dt.bfloat16))
            for i in range(2)]
        win_ts = [ctx.enter_context(
            nc.sbuf_tensor(f"win{i}", [128, maxw * D], mybir.dt.bfloat16))
            for i in range(2)]
        prod_t = ctx.enter_context(
            nc.sbuf_tensor([128, maxw * D], mybir.dt.bfloat16))
        red_ts = [ctx.enter_context(
            nc.sbuf_tensor(f"red{i}", [128, NRAYS_G], mybir.dt.float32))
            for i in range(2)]
        sel_t = ctx.enter_context(nc.sbuf_tensor([128, 16], mybir.dt.float32))
        vscr_t = ctx.enter_context(nc.sbuf_tensor([128, 2], mybir.dt.float32))
        ascr_t = ctx.enter_context(nc.sbuf_tensor([16, 2], mybir.dt.float32))
        sino_t = ctx.enter_context(
            nc.sbuf_tensor("sino", [16, SLOTS * NRAYS_G], mybir.dt.float32))
        psum_ts = [ctx.enter_context(
            nc.psum_tensor(f"ps{i}", [16, NRAYS_G], mybir.dt.float32))
            for i in range(2)]
        s_in = ctx.enter_context(nc.semaphore("s_in"))
        s_dma = ctx.enter_context(nc.semaphore("s_dma"))
        s_g = ctx.enter_context(nc.semaphore("s_g"))
        s_v = ctx.enter_context(nc.semaphore("s_v"))
        s_mm = ctx.enter_context(nc.semaphore("s_mm"))
        s_cp = ctx.enter_context(nc.semaphore("s_cp"))
        s_od = ctx.enter_context(nc.semaphore("s_od"))
        block = ctx.enter_context(nc.Block())

        # chunk schedule: (slot, cidx, xi0, nxi, cn, idx col offset, w offset)
        chunks = []
        for si in range(SLOTS):
            xi0 = 0
            ow = si * 2 * NTOT * D
            for cidx, nxi in enumerate(CHUNK_NXI):
                cn = nxi * NMAX
                chunks.append(dict(
                    si=si, cidx=cidx, xi0=xi0, nxi=nxi, cn=cn,
                    o16=si * (NTOT // 16) + xi0 * NMAX // 16,
                    ow=ow))
                ow += 2 * cn * D
                xi0 += nxi

        @block.sync
        def _(sync):
            slab_src = (slab_d[:].unsqueeze(0)
                        .broadcast_to([8, 16, NELEM * D]))
            sync.dma_start(out=slab_t[:], in_=slab_src).then_inc(s_in, 16)
            sync.dma_start(out=idx_t[:], in_=idx_d[:]).then_inc(s_in, 16)
            sync.dma_start(out=sel_t[:], in_=sel_d[:]).then_inc(s_in, 16)
            for n, ch in enumerate(chunks):
                if n > 1:
                    sync.wait_ge(s_v, n - 1)  # wt buffer consumed
                wsrc = (w_d[:, ch["ow"]:ch["ow"] + 2 * ch["cn"] * D]
                        .unsqueeze(1)
                        .broadcast_to([64, 2, 2 * ch["cn"] * D]))
                sync.dma_start(out=wt_ts[n % 2][:, :2 * ch["cn"] * D],
                               in_=wsrc).then_inc(s_dma, 16)

        @block.gpsimd
        def _(g):
            g.load_library(library_config.ap_gather)
            g.wait_ge(s_in, 48)
            g.wait_ge(s_dma, 16)
            for _ in range(2):  # warmup (IRAM load) + DMA landing barrier
                g.ap_gather(
                    win_ts[0][:, :64 * D].rearrange(
                        "p (n d) -> p n d", d=D),
                    slab_t[:].rearrange("p (n d) -> p n d", d=D),
                    idx_t[:, :4],
                    channels=128, num_elems=NELEM, d=D, num_idxs=64,
                )
            for n, ch in enumerate(chunks):
                if n > 1:
                    g.wait_ge(s_v, n - 1)  # win buffer consumed
                g.ap_gather(
                    win_ts[n % 2][:, :ch["cn"] * D].rearrange(
                        "p (n d) -> p n d", d=D),
                    slab_t[:].rearrange("p (n d) -> p n d", d=D),
                    idx_t[:, ch["o16"]:ch["o16"] + ch["cn"] // 16],
                    channels=128, num_elems=NELEM, d=D, num_idxs=ch["cn"],
                ).then_inc(s_g, 1)

        @block.vector
        def _(v):
            for n, ch in enumerate(chunks):
                v.wait_ge(s_g, n + 1)
                v.wait_ge(s_dma, 16 * (n + 1))
                if ch["cidx"] == 0 and ch["si"] > 1:
                    v.wait_ge(s_mm, ch["si"] - 1)  # red buffer consumed
                red = red_ts[ch["si"] % 2]
                for e in (0, 1):
                    v.tensor_mul(
                        prod_t[:, :ch["cn"] * D],
                        win_ts[n % 2][:, :ch["cn"] * D],
                        wt_ts[n % 2][:, e * ch["cn"] * D:
                                     (e + 1) * ch["cn"] * D])
                    v.tensor_reduce(
                        out=red[:, e * NXI + ch["xi0"]:
                                e * NXI + ch["xi0"] + ch["nxi"]],
                        in_=prod_t[:, :ch["cn"] * D].rearrange(
                            "p (x l) -> p x l", l=SEG),
                        axis=mybir.AxisListType.X,
                        op=mybir.AluOpType.add,
                    )
                # drain fence: DVE in-order; this inc observes the reduces
                v.tensor_copy(
                    vscr_t[:, :1],
                    red[:, NXI + ch["xi0"]:NXI + ch["xi0"] + 1],
                ).then_inc(s_v, 1)

        @block.tensor
        def _(t):
            for si in range(SLOTS):
                t.wait_ge(s_v, (si + 1) * NCHUNK)
                if si > 1:
                    t.wait_ge(s_cp, si - 1)  # psum consumed by scalar copy
                t.matmul(psum_ts[si % 2][:], sel_t[:], red_ts[si % 2][:],
                         start=True, stop=True).then_inc(s_mm, 1)

        @block.scalar
        def _(sc):
            for si in range(SLOTS):
                sc.wait_ge(s_mm, si + 1)
                sc.copy(sino_t[:, si * NRAYS_G:(si + 1) * NRAYS_G],
                        psum_ts[si % 2][:])
                sc.copy(ascr_t[:, :1],
                        sino_t[:, si * NRAYS_G:si * NRAYS_G + 1]
                        ).then_inc(s_cp, 1)
            sc.wait_ge(s_cp, SLOTS)
            sc.dma_start(out=out_d[:], in_=sino_t[:]).then_inc(s_od, 16)
            sc.wait_ge(s_od, 16)

    mybir.codegen_inst_isa_subclasses(nc)
    _PROG_CACHE["prog"] = nc
    return nc


def kernel(image):
    image = np.asarray(image, np.float32)
    assert image.shape == (BATCH, 1, IMG_SIZE, IMG_SIZE)
    plan = _get_plan()
    nc = _build_program()

    from concourse.bass_utils import run_bass_kernel_spmd

    slabs = {rep: _build_slab(image, rep) for rep in (0, 1)}
    in_maps = []
    for ci in range(8):
        in_maps.append({
            "slab": slabs[CORE_REP[ci]],
            "idx": plan["core_idx"][ci],
            "w": plan["core_w"][ci],
            "sel": plan["sel"],
        })

    trace = bool(os.environ.get("RADON_TRACE"))
    if trace:
        _install_profhook()
    res = run_bass_kernel_spmd(nc, in_maps, list(range(8)), trace=trace)
    if trace:
        kernel.last_exec_time_ns = res.exec_time_ns

    sino = np.zeros((BATCH, 1, S, N_ANGLES), np.float32)
    for ci in range(8):
        o = res.results[ci]["out"]  # [16, SLOTS*NRAYS_G]
        for si, k in enumerate(CORE_ANGLES[ci]):
            v = (o[:, si * NRAYS_G:(si + 1) * NRAYS_G]
                 .reshape(8, 2, 2, NXI))  # [g, b, e, xi]
            # x = 16*xi + 2g + e
            full = v.transpose(1, 3, 0, 2).reshape(BATCH, NXI * 16)
            sino[:, 0, :, k] = full[:, :S]
    return sino


def _install_profhook():
    import types
    if "antenv.axon_hooks" in sys.modules:
        return
    try:
        from trn_agent_boot.trn_boot import _ntff_profile_via_ctypes
        hook = _ntff_profile_via_ctypes("/opt/axon/libaxon_pjrt.so")
    except Exception:
        hook = None
    mod = types.ModuleType("antenv.axon_hooks")
    mod._hook = hook
    mod.set_axon_ntff_profile_hook = lambda h: setattr(mod, "_hook", h)
    mod.get_axon_ntff_profile_hook = lambda: mod._hook
    sys.modules["antenv.axon_hooks"] = mod
    import antenv
    antenv.axon_hooks = mod


if __name__ == "__main__":
    img = np.load("/tmp/ref_image.npy")
    out = kernel(image=img)
    exp = np.load("/tmp/ref_expected.npy")
    err = np.linalg.norm(out - exp) / np.linalg.norm(exp)
    print("kernel rel err:", err)


# revision 16
# speedup vs baseline: 13.7684x; 1.2332x over previous
"""Radon transform (bilinear grid-sample + row-sum) on 8 TRN2 NeuronCores.

Angle wedges sharded across 8 cores (rep-pure wedges: identity frame for
|cos|>=|sin|, transposed frame otherwise). Per core, per angle: every
detector ray x is decomposed into 32 8-row blocks of the content region;
one GPSIMD ap_gather index per (ray, block) fetches a 16-wide column window
(hop-4 aligned, overlapping slab storage) that covers all bilinear taps of
the ray in that block. The 16 channels of each Q7 core hold the 8 row
phases x 2 batches, so every gathered lane is useful. DVE multiplies by
precomputed tap weights and segment-reduces each ray (32 blocks x 16 taps
= 512 elems); a TensorE ones-style matmul sums the 128 partitions into
(group, batch) sinogram rows. All indices/weights are input-independent
and precomputed on host.
"""
import math
import os
import sys
from contextlib import ExitStack

import numpy as np

sys.path.insert(0, "/opt/trn_rl_repo")

import ml_dtypes  # noqa: E402

BF16 = ml_dtypes.bfloat16

# ─── geometry constants (hardcoded for 256x256, 180 angles, batch 2) ───
N_ANGLES = 180
IMG_SIZE = 256
BATCH = 2
S = int(math.ceil(math.sqrt(2.0) * IMG_SIZE))  # 363
PB = (S - IMG_SIZE) // 2                       # 53
ROFF = 53         # slab row/col origin = content origin
HOP = 4           # window alignment granularity
D = 16            # window width (bf16 elems per gather block)
NH = 64           # hop positions per slab row
NJ = 32           # 8-row blocks covering the 256 content rows
NELEM = NJ * NH   # 2048 gather blocks per slab partition
NMAX = NJ         # block slots per ray-pair (j used directly)
NXI = 23          # ray-pairs per Q7 group (x = 16*xi + 2g + e)
NRAYS_G = 2 * NXI              # 46 ray columns per group (e-major)
NTOT = NXI * NMAX              # 736 indices per slot per group
SEG = NMAX * D                 # 512 elems reduced per (ray, e)
SLOTS = 23
CHUNK_NXI = [12, 11]           # ray-pairs per chunk
NCHUNK = len(CHUNK_NXI)

CORE_ANGLES = [
    list(range(0, 23)), list(range(23, 46)),
    list(range(135, 158)), list(range(158, 180)),
    list(range(46, 69)), list(range(69, 91)),
    list(range(91, 113)), list(range(113, 135)),
]
CORE_REP = [0, 0, 0, 0, 1, 1, 1, 1]


def _angle_taps(k):
    """Content-region bilinear taps in rep-frame coords.

    Returns rep, xs (detector ray), j (8-row block), phi (row phase),
    cc (slab col = col-ROFF), ws (f32 weight)."""
    th = np.float32(k) * np.float32(np.pi / N_ANGLES)
    c = np.cos(th, dtype=np.float32)
    s = np.sin(th, dtype=np.float32)
    lin = np.linspace(-1.0, 1.0, S, dtype=np.float32)
    gx = c * lin[None, :] + s * lin[:, None]
    gy = -s * lin[None, :] + c * lin[:, None]
    ix = (gx + np.float32(1)) * np.float32(0.5) * np.float32(S - 1)
    iy = (gy + np.float32(1)) * np.float32(0.5) * np.float32(S - 1)
    x0 = np.floor(ix).astype(np.int64)
    y0 = np.floor(iy).astype(np.int64)
    wx = (ix - x0).astype(np.float32)
    wy = (iy - y0).astype(np.float32)
    rep = 0 if abs(c) >= abs(s) else 1
    rows_l, cols_l, ws_l, xs_l = [], [], [], []
    for dy in (0, 1):
        for dx in (0, 1):
            r = y0 + dy
            q = x0 + dx
            w = (wy if dy else 1 - wy) * (wx if dx else 1 - wx)
            m = ((r >= PB) & (r < PB + IMG_SIZE)
                 & (q >= PB) & (q < PB + IMG_SIZE) & (w != 0))
            _, xx = np.nonzero(m)
            rows_l.append(r[m])
            cols_l.append(q[m])
            ws_l.append(w[m])
            xs_l.append(xx)
    rows = np.concatenate(rows_l)
    cols = np.concatenate(cols_l)
    ws = np.concatenate(ws_l)
    xs = np.concatenate(xs_l)
    if rep:
        rows, cols = cols, rows
    j = (rows - ROFF) // 8
    phi = (rows - ROFF) % 8
    cc = cols - ROFF
    return rep, xs, j, phi, cc, ws


def _plan_angle(k):
    """Pair layout: pair p2 = x//2 (g = p2%8, xi = p2//8), e = x%2.

    Returns idx [184, NJ] int16 block ids per pair, and
    wt [2, 184, NJ, 8, D] f32 weights (e-major)."""
    rep, xs, j, phi, cc, ws = _angle_taps(k)
    p2 = xs // 2
    e = xs % 2
    qmin = np.full((184, NJ), 10 ** 6, np.int64)
    np.minimum.at(qmin, (p2, j), cc)
    hq = np.clip(qmin // HOP, 0, NH - D // HOP)
    wt = np.zeros((2, 184, NJ, 8, D), np.float32)
    kk = cc - HOP * hq[p2, j]
    assert kk.min() >= 0 and kk.max() < D, (k, kk.min(), kk.max())
    np.add.at(wt, (e, p2, j, phi, kk), ws)
    present = np.zeros((184, NJ), bool)
    present[p2, j] = True
    idx = np.where(present, np.arange(NJ)[None, :] * NH + hq, 0)
    return rep, idx.astype(np.int16), wt


_PLAN_CACHE = {}


def _get_plan():
    if "plan" in _PLAN_CACHE:
        return _PLAN_CACHE["plan"]
    slot_w = SLOTS * 2 * NTOT * D
    core_idx = []
    core_w = []
    for ci in range(8):
        idx_blob = np.zeros((128, SLOTS * (NTOT // 16)), np.int16)
        w_blob = np.zeros((64, slot_w // SLOTS * SLOTS), np.float32)
        for si, k in enumerate(CORE_ANGLES[ci]):
            rep, idx, wt = _plan_angle(k)
            assert rep == CORE_REP[ci]
            # idx[p2, j] -> group g = p2%8, n = xi*NJ + j
            ig = idx.reshape(NXI, 8, NJ).transpose(1, 0, 2).reshape(8, NTOT)
            wrap = ig.reshape(8, NTOT // 16, 16)
            for g in range(8):
                idx_blob[16 * g:16 * g + 16,
                         si * (NTOT // 16):(si + 1) * (NTOT // 16)] = wrap[g].T
            # wt[e, p2, j, phi, k] -> row 8g+phi,
            # chunk-major cols: [chunk][e][xi_local][j][k]
            wg = (wt.reshape(2, NXI, 8, NJ, 8, D)
                  .transpose(2, 4, 0, 1, 3, 5))  # [g, phi, e, xi, j, k]
            base = si * 2 * NTOT * D
            xi0 = 0
            for nxi in CHUNK_NXI:
                sz = 2 * nxi * NJ * D
                blockw = (wg[:, :, :, xi0:xi0 + nxi]
                          .reshape(64, sz))
                w_blob[:, base:base + sz] = blockw
                base += sz
                xi0 += nxi
        core_idx.append(idx_blob)
        core_w.append(w_blob.astype(BF16))
    sel = np.zeros((128, 16), np.float32)
    for p in range(128):
        sel[p, 2 * (p // 16) + (p % 2)] = 1.0
    plan = dict(core_idx=core_idx, core_w=core_w, sel=sel)
    _PLAN_CACHE["plan"] = plan
    return plan


def _build_slab(image, rep):
    """[128, NELEM*D] bf16: channel p%16 = 2*phi+b holds hop-4 overlapping
    windows of content rows ROFF+8j+phi (replicated across the 8 groups)."""
    fr = np.zeros((BATCH, S, S), np.float32)
    fr[:, PB:PB + IMG_SIZE, PB:PB + IMG_SIZE] = image[:, 0]
    if rep:
        fr = np.ascontiguousarray(np.transpose(fr, (0, 2, 1)))
    out = np.zeros((16, NELEM * D), np.float32)
    span = HOP * (NH - 1) + D  # 268 cols
    for phi in range(8):
        rows = fr[:, ROFF + phi: ROFF + phi + 8 * NJ: 8, ROFF:ROFF + span]
        win = np.lib.stride_tricks.sliding_window_view(rows, D, axis=2)
        win = win[:, :, ::HOP, :]  # [B, NJ, NH, D]
        assert win.shape == (BATCH, NJ, NH, D)
        for b in range(BATCH):
            out[2 * phi + b] = win[b].reshape(-1)
    out16 = out.astype(BF16)
    return np.ascontiguousarray(np.broadcast_to(
        out16[None], (8, 16, NELEM * D)).reshape(128, NELEM * D))


_PROG_CACHE = {}


def _build_program():
    if "prog" in _PROG_CACHE:
        return _PROG_CACHE["prog"]
    import concourse.bass as bass
    import concourse.mybir as mybir
    from concourse import library_config

    nc = bass.Bass()
    slab_d = nc.declare_dram_parameter("slab", [128, NELEM * D],
                                       mybir.dt.bfloat16, isOutput=False)
    idx_d = nc.declare_dram_parameter("idx", [128, SLOTS * (NTOT // 16)],
                                      mybir.dt.int16, isOutput=False)
    w_d = nc.declare_dram_parameter("w", [64, SLOTS * 2 * NTOT * D],
                                    mybir.dt.bfloat16, isOutput=False)
    sel_d = nc.declare_dram_parameter("sel", [128, 16], mybir.dt.float32,
                                      isOutput=False)
    out_d = nc.declare_dram_parameter("out", [16, SLOTS * NRAYS_G],
                                      mybir.dt.float32, isOutput=True)

    ctx = ExitStack()
    with ctx:
        slab_t = ctx.enter_context(
            nc.sbuf_tensor([128, NELEM * D], mybir.dt.bfloat16))
        idx_t = ctx.enter_context(
            nc.sbuf_tensor([128, SLOTS * (NTOT // 16)], mybir.dt.int16))
        maxw = max(CHUNK_NXI) * NJ  # windows per chunk
        wt_ts = [ctx.enter_context(
            nc.sbuf_tensor(f"wt{i}", [128, 2 * maxw * D], mybir.dt.bfloat16))
            for i in range(2)]
        win_ts = [ctx.enter_context(
            nc.sbuf_tensor(f"win{i}", [128, maxw * D], mybir.dt.bfloat16))
            for i in range(2)]
        prod_t = ctx.enter_context(
            nc.sbuf_tensor([128, maxw * D], mybir.dt.bfloat16))
        red1_t = ctx.enter_context(
            nc.sbuf_tensor([128, maxw], mybir.dt.bfloat16))
        red_ts = [ctx.enter_context(
            nc.sbuf_tensor(f"red{i}", [128, NRAYS_G], mybir.dt.float32))
            for i in range(2)]
        sel_t = ctx.enter_context(nc.sbuf_tensor([128, 16], mybir.dt.float32))
        vscr_t = ctx.enter_context(nc.sbuf_tensor([128, 2], mybir.dt.float32))
        ascr_t = ctx.enter_context(nc.sbuf_tensor([16, 2], mybir.dt.float32))
        sino_t = ctx.enter_context(
            nc.sbuf_tensor("sino", [16, SLOTS * NRAYS_G], mybir.dt.float32))
        psum_ts = [ctx.enter_context(
            nc.psum_tensor(f"ps{i}", [16, NRAYS_G], mybir.dt.float32))
            for i in range(2)]
        s_in = ctx.enter_context(nc.semaphore("s_in"))
        s_dma = ctx.enter_context(nc.semaphore("s_dma"))
        s_g = ctx.enter_context(nc.semaphore("s_g"))
        s_v = ctx.enter_context(nc.semaphore("s_v"))
        s_mm = ctx.enter_context(nc.semaphore("s_mm"))
        s_cp = ctx.enter_context(nc.semaphore("s_cp"))
        s_od = ctx.enter_context(nc.semaphore("s_od"))
        block = ctx.enter_context(nc.Block())

        # chunk schedule: (slot, cidx, xi0, nxi, cn, idx col offset, w offset)
        chunks = []
        for si in range(SLOTS):
            xi0 = 0
            ow = si * 2 * NTOT * D
            for cidx, nxi in enumerate(CHUNK_NXI):
                cn = nxi * NMAX
                chunks.append(dict(
                    si=si, cidx=cidx, xi0=xi0, nxi=nxi, cn=cn,
                    o16=si * (NTOT // 16) + xi0 * NMAX // 16,
                    ow=ow))
                ow += 2 * cn * D
                xi0 += nxi

        @block.sync
        def _(sync):
            sync.dma_start(out=slab_t[:], in_=slab_d[:]).then_inc(s_in, 16)
            sync.dma_start(out=idx_t[:], in_=idx_d[:]).then_inc(s_in, 16)
            sync.dma_start(out=sel_t[:], in_=sel_d[:]).then_inc(s_in, 16)
            for n, ch in enumerate(chunks):
                if n > 1:
                    sync.wait_ge(s_v, n - 1)  # wt buffer consumed
                wsrc = (w_d[:, ch["ow"]:ch["ow"] + 2 * ch["cn"] * D]
                        .unsqueeze(1)
                        .broadcast_to([64, 2, 2 * ch["cn"] * D]))
                sync.dma_start(out=wt_ts[n % 2][:, :2 * ch["cn"] * D],
                               in_=wsrc).then_inc(s_dma, 16)

        @block.gpsimd
        def _(g):
            g.load_library(library_config.ap_gather)
            g.wait_ge(s_in, 48)
            g.wait_ge(s_dma, 16)
            for _ in range(2):  # warmup (IRAM load) + DMA landing barrier
                g.ap_gather(
                    win_ts[0][:, :64 * D].rearrange(
                        "p (n d) -> p n d", d=D),
                    slab_t[:].rearrange("p (n d) -> p n d", d=D),
                    idx_t[:, :4],
                    channels=128, num_elems=NELEM, d=D, num_idxs=64,
                )
            for n, ch in enumerate(chunks):
                if n > 1:
                    g.wait_ge(s_v, n - 1)  # win buffer consumed
                g.ap_gather(
                    win_ts[n % 2][:, :ch["cn"] * D].rearrange(
                        "p (n d) -> p n d", d=D),
                    slab_t[:].rearrange("p (n d) -> p n d", d=D),
                    idx_t[:, ch["o16"]:ch["o16"] + ch["cn"] // 16],
                    channels=128, num_elems=NELEM, d=D, num_idxs=ch["cn"],
                ).then_inc(s_g, 1)

        @block.vector
        def _(v):
            for n, ch in enumerate(chunks):
                v.wait_ge(s_g, n + 1)
                v.wait_ge(s_dma, 16 * (n + 1))
                if ch["cidx"] == 0 and ch["si"] > 1:
                    v.wait_ge(s_mm, ch["si"] - 1)  # red buffer consumed
                red = red_ts[ch["si"] % 2]
                for e in (0, 1):
                    v.tensor_mul(
                        prod_t[:, :ch["cn"] * D],
                        win_ts[n % 2][:, :ch["cn"] * D],
                        wt_ts[n % 2][:, e * ch["cn"] * D:
                                     (e + 1) * ch["cn"] * D])
                    # stage 1: per-window sums in bf16 (2x-mode eligible)
                    with nc.allow_low_precision("16-elem window partials"):
                        v.tensor_reduce(
                            out=red1_t[:, :ch["cn"]],
                            in_=prod_t[:, :ch["cn"] * D].rearrange(
                                "p (x l) -> p x l", l=D),
                            axis=mybir.AxisListType.X,
                            op=mybir.AluOpType.add,
                        )
                    # stage 2: per-ray sums over NMAX blocks into f32
                    v.tensor_reduce(
                        out=red[:, e * NXI + ch["xi0"]:
                                e * NXI + ch["xi0"] + ch["nxi"]],
                        in_=red1_t[:, :ch["cn"]].rearrange(
                            "p (x l) -> p x l", l=NMAX),
                        axis=mybir.AxisListType.X,
                        op=mybir.AluOpType.add,
                    )
                # drain fence: DVE in-order; this inc observes the reduces
                v.tensor_copy(
                    vscr_t[:, :1],
                    red[:, NXI + ch["xi0"]:NXI + ch["xi0"] + 1],
                ).then_inc(s_v, 1)

        @block.tensor
        def _(t):
            for si in range(SLOTS):
                t.wait_ge(s_v, (si + 1) * NCHUNK)
                if si > 1:
                    t.wait_ge(s_cp, si - 1)  # psum consumed by scalar copy
                t.matmul(psum_ts[si % 2][:], sel_t[:], red_ts[si % 2][:],
                         start=True, stop=True).then_inc(s_mm, 1)

        @block.scalar
        def _(sc):
            for si in range(SLOTS):
                sc.wait_ge(s_mm, si + 1)
                sc.copy(sino_t[:, si * NRAYS_G:(si + 1) * NRAYS_G],
                        psum_ts[si % 2][:])
                sc.copy(ascr_t[:, :1],
                        sino_t[:, si * NRAYS_G:si * NRAYS_G + 1]
                        ).then_inc(s_cp, 1)
            sc.wait_ge(s_cp, SLOTS)
            sc.dma_start(out=out_d[:], in_=sino_t[:]).then_inc(s_od, 16)
            sc.wait_ge(s_od, 16)

    mybir.codegen_inst_isa_subclasses(nc)
    _PROG_CACHE["prog"] = nc
    return nc


def kernel(image):
    image = np.asarray(image, np.float32)
    assert image.shape == (BATCH, 1, IMG_SIZE, IMG_SIZE)
    plan = _get_plan()
    nc = _build_program()

    from concourse.bass_utils import run_bass_kernel_spmd

    slabs = {rep: _build_slab(image, rep) for rep in (0, 1)}
    in_maps = []
    for ci in range(8):
        in_maps.append({
            "slab": slabs[CORE_REP[ci]],
            "idx": plan["core_idx"][ci],
            "w": plan["core_w"][ci],
            "sel": plan["sel"],
        })

    trace = bool(os.environ.get("RADON_TRACE"))
    if trace:
        _install_profhook()
    res = run_bass_kernel_spmd(nc, in_maps, list(range(8)), trace=trace)
    if trace:
        kernel.last_exec_time_ns = res.exec_time_ns

    sino = np.zeros((BATCH, 1, S, N_ANGLES), np.float32)
    for ci in range(8):
        o = res.results[ci]["out"]  # [16, SLOTS*NRAYS_G]
        for si, k in enumerate(CORE_ANGLES[ci]):
            v = (o[:, si * NRAYS_G:(si + 1) * NRAYS_G]
                 .reshape(8, 2, 2, NXI))  # [g, b, e, xi]
            # x = 16*xi + 2g + e
            full = v.transpose(1, 3, 0, 2).reshape(BATCH, NXI * 16)
            sino[:, 0, :, k] = full[:, :S]
    return sino


def _install_profhook():
    import types
    if "antenv.axon_hooks" in sys.modules:
        return
    try:
        from trn_agent_boot.trn_boot import _ntff_profile_via_ctypes
        hook = _ntff_profile_via_ctypes("/opt/axon/libaxon_pjrt.so")
    except Exception:
        hook = None
    mod = types.ModuleType("antenv.axon_hooks")
    mod._hook = hook
    mod.set_axon_ntff_profile_hook = lambda h: setattr(mod, "_hook", h)
    mod.get_axon_ntff_profile_hook = lambda: mod._hook
    sys.modules["antenv.axon_hooks"] = mod
    import antenv
    antenv.axon_hooks = mod


if __name__ == "__main__":
    img = np.load("/tmp/ref_image.npy")
    out = kernel(image=img)
    exp = np.load("/tmp/ref_expected.npy")
    err = np.linalg.norm(out - exp) / np.linalg.norm(exp)
    print("kernel rel err:", err)
